# revision 1
# baseline (speedup 1.0000x reference)
"""Causal multi-head attention (B=256, T=197, C=768, H=12, D=64) on 8 trn2 cores.

Strategy:
- Data-parallel over batch: 32 batches per core, no collectives.
- Host pre-transposes x to [C, T] layout per batch (xT), so Q^T/K^T come out of
  the projection matmuls directly in [C, T] layout and V in [T, C] layout.
- Per (batch, head): S^T = K^T_slice.T @ Q^T  ([k, q] layout, f32 PSUM),
  P^T = exp(S^T) (ACT, no max-subtraction: scores are bounded ~|2|),
  causal mask applied as a multiplicative 0/1 bf16 mask (DVE) on the
  triangle region only (rectangle decomposition: the k>=128 block only
  computes q>=128).
- ctx^T = [V | 1].T @ P^T : the appended ones-column makes row 64 of the
  PSUM output the softmax denominator for free.
- Normalize at eviction: reciprocal of the sums row (DVE), broadcast across
  partitions via a DRAM-tile bounce (DRAM APs allow partition step-0),
  multiply during PSUM->SBUF eviction (DVE), giving ctx^T bf16.
- out = ctx^T.T @ Wo via matmul with rhs=Wo, evict f32, DMA out.
- All matmuls bf16 (1 cyc/row on PE); accumulation f32 in PSUM; softmax
  internals f32; final output f32.
"""

import numpy as np

B, T, C, H = 256, 197, 768, 12
D = C // H          # 64
P = 128             # partition size
CB = C // P         # 6 c-blocks
NCORES = 8
NB = B // NCORES    # 32 batches per core
G = 4               # batches per projection group
NG = NB // G        # 8 groups
TG = G * T          # 788 tokens per group
T0 = P              # first t/k block rows (128)
T1 = T - P          # second block rows (69)

_CACHE = {}


def _split_ctrl_waits(nc):
    """This walrus encodes at most 1 sem wait per instruction (2 for
    EventSemaphore), but Tile emits instructions with several. Split excess
    waits onto NoOps inserted before the offending instruction on the same
    engine (a NoOp itself carries 1 wait)."""
    import concourse.mybir as mybir

    for fn in nc.m.functions:
        for bb in fn.blocks:
            insts = bb.instructions
            newlist = []
            changed = False
            for inst in insts:
                cap = 2 if isinstance(inst, mybir.InstEventSemaphore) else 1
                si = inst.sync_info
                waits = list(si.on_wait) if si and si.on_wait else []
                if len(waits) > cap:
                    changed = True
                    head, rest = waits[:-cap], waits[-cap:]
                    for w in head:
                        nop = mybir.InstNoOp(
                            name=nc.get_next_instruction_name(),
                            bass_nofuse=True,
                            engine=inst.engine,
                            sync_info=mybir.SyncInfo(on_wait=[w], on_update=[]),
                        )
                        newlist.append(nop)
                    inst.sync_info = mybir.SyncInfo(
                        on_wait=rest,
                        on_update=list(si.on_update) if si.on_update else [],
                    )
                newlist.append(inst)
            if changed:
                bb.instructions = newlist


def _dedup_ldweights(nc):
    """Delete an InstLdweights that reloads the exact weights already loaded
    by the previous PE ldweights with no different load in between (our
    512/276-column chunk pairs share lhsT). Only drops wait-free duplicates."""
    import concourse.mybir as mybir

    ndrop = 0
    for fn in nc.m.functions:
        for bb in fn.blocks:
            insts = bb.instructions
            newlist = []
            last_sig = None
            changed = False
            for inst in insts:
                if inst.engine != mybir.EngineType.PE:
                    newlist.append(inst)
                    continue
                if type(inst).__name__ == "InstLdweights":
                    si = inst.sync_info
                    nw = len(si.on_wait) if si and si.on_wait else 0
                    nu = len(si.on_update) if si and si.on_update else 0
                    sig = (str(inst.ins[0]), str(inst.tile_position),
                           str(inst.tile_size), str(inst.is_transpose),
                           str(inst.perf_mode))
                    if sig == last_sig and nw == 0 and nu == 0:
                        changed = True
                        ndrop += 1
                        continue  # drop duplicate
                    last_sig = sig
                newlist.append(inst)
            if changed:
                bb.instructions = newlist
    return ndrop


def build_nc(nb=NB, split_waits=True, repeat=1):
    import concourse.bass as bass
    import concourse.mybir as mybir
    from concourse.tile import TileContext

    f32 = mybir.dt.float32
    bf16 = mybir.dt.bfloat16
    Exp = mybir.ActivationFunctionType.Exp
    Copy = mybir.ActivationFunctionType.Copy

    ng = nb // G

    nc = bass.Bass()
    xT = nc.declare_dram_parameter("xT", [CB, P, nb * T], bf16, isOutput=False)
    wq = nc.declare_dram_parameter("wq", [CB, P, C], bf16, isOutput=False)
    wk = nc.declare_dram_parameter("wk", [CB, P, C], bf16, isOutput=False)
    wv = nc.declare_dram_parameter("wv", [CB, P, C], bf16, isOutput=False)
    wo = nc.declare_dram_parameter("wo", [CB, P, C], bf16, isOutput=False)
    maskp = nc.declare_dram_parameter("mask", [P, T], bf16, isOutput=False)
    out = nc.declare_dram_parameter("out", [nb, T, C], f32, isOutput=True)

    with TileContext(nc) as tc:
        with (
            tc.tile_pool(name="weights", bufs=1) as wpool,
            tc.tile_pool(name="x", bufs=2) as xpool,
            tc.tile_pool(name="qk", bufs=2) as qkpool,
            tc.tile_pool(name="v", bufs=8) as vpool,
            tc.tile_pool(name="p0", bufs=4) as p0pool,
            tc.tile_pool(name="p1", bufs=4) as p1pool,
            tc.tile_pool(name="inv", bufs=6) as invpool,
            tc.tile_pool(name="invbc", bufs=6) as invbcpool,
            tc.tile_pool(name="stage", bufs=6) as stagepool,
            tc.tile_pool(name="ctxt", bufs=2) as ctxtpool,
            tc.tile_pool(name="outsb", bufs=3) as outsbpool,
            tc.tile_pool(name="invdram", bufs=8, space="DRAM") as invdram,
            tc.tile_pool(name="bigps", bufs=2, space="PSUM") as bigps,
            tc.tile_pool(name="sps", bufs=2, space="PSUM") as sps,
            tc.tile_pool(name="ctxps", bufs=2, space="PSUM") as ctxps,
        ):
            # --- static tiles ---
            wq_sb = wpool.tile([P, CB, C], bf16, tag="wq")
            wk_sb = wpool.tile([P, CB, C], bf16, tag="wk")
            wv_sb = wpool.tile([P, CB, C], bf16, tag="wv")
            wo_sb = wpool.tile([P, CB, C], bf16, tag="wo")
            mask_sb = wpool.tile([P, T], bf16, tag="mask")
            for dram, sb in ((wq, wq_sb), (wk, wk_sb), (wv, wv_sb), (wo, wo_sb)):
                nc.sync.dma_start(out=sb[:], in_=dram.rearrange("ib p c -> p ib c"))
            nc.sync.dma_start(out=mask_sb[:], in_=maskp[:])

            for g in range(ng * repeat):
                g = g % ng
                xT_sb = xpool.tile([P, CB, TG], bf16, tag="xT")
                nc.sync.dma_start(
                    out=xT_sb[:],
                    in_=xT[:, :, g * TG:(g + 1) * TG].rearrange("ib p t -> p ib t"),
                )

                # --- q^T / k^T projections, [c, t] layout, 4 batches at once ---
                qT_sb = qkpool.tile([P, CB, TG], bf16, tag="qT")
                kT_sb = qkpool.tile([P, CB, TG], bf16, tag="kT")
                for w_sb, dst in ((wq_sb, qT_sb), (wk_sb, kT_sb)):
                    for cb in range(CB):
                        ps = bigps.tile([P, TG], f32, tag="ps")
                        for ib in range(CB):
                            lhs = w_sb[:, ib, cb * P:(cb + 1) * P]
                            nc.tensor.matmul(
                                ps[:, 0:512], lhsT=lhs, rhs=xT_sb[:, ib, 0:512],
                                start=(ib == 0), stop=(ib == CB - 1))
                            nc.tensor.matmul(
                                ps[:, 512:TG], lhsT=lhs,
                                rhs=xT_sb[:, ib, 512:TG],
                                start=(ib == 0), stop=(ib == CB - 1))
                        nc.scalar.activation(dst[:, cb, :], ps[:, :], Copy)

                # --- v projection, [t, c] layout, per batch ---
                vtiles = {}
                for b in range(G):
                    for tb in range(2):
                        rows = T0 if tb == 0 else T1
                        col0 = b * T + tb * P
                        ps = bigps.tile([P, TG], f32, tag="ps")
                        for ib in range(CB):
                            lhs = xT_sb[:, ib, col0:col0 + rows]
                            nc.tensor.matmul(
                                ps[0:rows, 0:512], lhsT=lhs, rhs=wv_sb[:, ib, 0:512],
                                start=(ib == 0), stop=(ib == CB - 1))
                            nc.tensor.matmul(
                                ps[0:rows, 512:C], lhsT=lhs,
                                rhs=wv_sb[:, ib, 512:C],
                                start=(ib == 0), stop=(ib == CB - 1))
                        v_sb = vpool.tile([P, H, D + 1], bf16, tag="v")
                        nc.scalar.activation(
                            v_sb[0:rows, :, 0:D],
                            ps[0:rows, 0:C].rearrange("p (h d) -> p h d", d=D), Copy)
                        nc.vector.memset(v_sb[0:rows, :, D], 1.0)
                        vtiles[(b, tb)] = v_sb

                # --- attention + output projection, per batch ---
                for b in range(G):
                    ctxT_sb = ctxtpool.tile([P, CB, T], bf16, tag="ctxT")
                    for h in range(H):
                        j, i = h // 2, h % 2
                        base = i * D
                        qh = qT_sb[base:base + D, j, b * T:(b + 1) * T]
                        kh = kT_sb[base:base + D, j, b * T:(b + 1) * T]
                        s0 = sps.tile([P, 272], f32, tag="s")
                        # S^T block0: k in [0,128), all q
                        nc.tensor.matmul(s0[:, 0:T], lhsT=kh[:, 0:P], rhs=qh,
                                         start=True, stop=True)
                        # S^T block1: k in [128,197), q in [128,197)
                        nc.tensor.matmul(s0[0:T1, 200:200 + T1],
                                         lhsT=kh[:, P:T], rhs=qh[:, P:T],
                                         start=True, stop=True)
                        p0 = p0pool.tile([P, T], bf16, tag="p0")
                        p1 = p1pool.tile([T1, T1], bf16, tag="p1")
                        nc.scalar.activation(p0[:], s0[:, 0:T], Exp)
                        nc.scalar.activation(p1[:], s0[0:T1, 200:200 + T1], Exp)
                        # causal mask: multiplicative 0/1 on the triangles
                        nc.vector.tensor_mul(p0[:, 0:P], p0[:, 0:P],
                                             mask_sb[:, 0:P])
                        nc.vector.tensor_mul(p1[:], p1[:], mask_sb[0:T1, 0:T1])
                        # ctx~^T (+ sums in row 64); per-head PSUM tile so the
                        # sums row sits at free-offset 0 (single-partition DVE
                        # PSUM reads at offset 256 are broken on this HW)
                        ctx_ps = ctxps.tile([D + 1, 256], f32, tag="ctx")
                        vb0 = vtiles[(b, 0)]
                        vb1 = vtiles[(b, 1)]
                        nc.tensor.matmul(ctx_ps[:, 0:T], lhsT=vb0[:, h, :],
                                         rhs=p0[:], start=True, stop=False,
                                         skip_group_check=True)
                        nc.tensor.matmul(ctx_ps[:, P:T], lhsT=vb1[0:T1, h, :],
                                         rhs=p1[:], start=False, stop=True,
                                         skip_group_check=True)
                        # normalize and place into ctxT rows [base, base+64)
                        inv_row = invpool.tile([1, T], f32, tag="inv")
                        inv_bc = invbcpool.tile([D, T], f32, tag="invbc")
                        nc.vector.reciprocal(inv_row[:], ctx_ps[D:D + 1, 0:T])
                        scr = invdram.tile([1, T], f32, tag="invscr")
                        nc.sync.dma_start(out=scr[:], in_=inv_row[:])
                        sv = scr[0]
                        inv_src = bass.AP(
                            tensor=sv.tensor,
                            offset=sv.offset,
                            ap=[[0, D]] + list(sv.ap),
                        )
                        nc.sync.dma_start(out=inv_bc[:], in_=inv_src)
                        bcols = slice(0, T)
                        if i == 0:
                            nc.vector.tensor_mul(ctxT_sb[0:D, j, bcols],
                                                 ctx_ps[0:D, 0:T], inv_bc[:])
                        else:
                            stage = stagepool.tile([D, T], bf16, tag="stage")
                            nc.vector.tensor_mul(stage[:], ctx_ps[0:D, 0:T],
                                                 inv_bc[:])
                            nc.sync.dma_start(out=ctxT_sb[D:P, j, bcols],
                                              in_=stage[:])

                    # --- out = ctx^T.T @ Wo ---
                    for tb in range(2):
                        rows = T0 if tb == 0 else T1
                        ps = bigps.tile([P, TG], f32, tag="ps")
                        for j in range(CB):
                            lhs = ctxT_sb[:, j, tb * P:tb * P + rows]
                            nc.tensor.matmul(
                                ps[0:rows, 0:512], lhsT=lhs, rhs=wo_sb[:, j, 0:512],
                                start=(j == 0), stop=(j == CB - 1))
                            nc.tensor.matmul(
                                ps[0:rows, 512:C], lhsT=lhs,
                                rhs=wo_sb[:, j, 512:C],
                                start=(j == 0), stop=(j == CB - 1))
                        out_sb = outsbpool.tile([P, C], f32, tag="out")
                        nc.scalar.activation(out_sb[0:rows, :], ps[0:rows, 0:C], Copy)
                        nc.sync.dma_start(
                            out=out[g * G + b, tb * P:tb * P + rows, :],
                            in_=out_sb[0:rows, :])

    _dedup_ldweights(nc)
    if split_waits:
        _split_ctrl_waits(nc)
    return nc


def _prep_core_inputs(hidden_states, Wq, Wk, Wv, Wo):
    """Host-side layout prep. Returns per-core in_maps (list of dicts)."""
    import ml_dtypes

    bf16 = ml_dtypes.bfloat16
    scale = 1.0 / np.sqrt(D)
    # xT[ib, p, b*T+t] = x[b, t, ib*128+p]
    x = np.ascontiguousarray(hidden_states.astype(np.float32))
    wq_h = np.ascontiguousarray((Wq * scale).reshape(CB, P, C).astype(bf16))
    wk_h = np.ascontiguousarray(Wk.reshape(CB, P, C).astype(bf16))
    wv_h = np.ascontiguousarray(Wv.reshape(CB, P, C).astype(bf16))
    wo_h = np.ascontiguousarray(Wo.reshape(CB, P, C).astype(bf16))
    mask = (np.arange(T)[None, :] >= np.arange(P)[:, None]).astype(bf16)

    in_maps = []
    for c in range(NCORES):
        xs = x[c * NB:(c + 1) * NB]  # [NB, T, C]
        xT = xs.reshape(NB, T, CB, P).transpose(2, 3, 0, 1).reshape(CB, P, NB * T)
        in_maps.append({
            "xT": np.ascontiguousarray(xT.astype(bf16)),
            "wq": wq_h, "wk": wk_h, "wv": wv_h, "wo": wo_h,
            "mask": mask,
        })
    return in_maps


def bench(inputs, iters=5):
    """Run the kernel with device-resident inputs; return (out, per-iter ns)."""
    import time

    if "nc" not in _CACHE:
        _CACHE["nc"] = build_nc()
    if "runner" not in _CACHE:
        _CACHE["runner"] = _make_runner(_CACHE["nc"])
    run, out_names, out_avals = _CACHE["runner"]
    in_maps = _prep_core_inputs(
        np.asarray(inputs["hidden_states"]),
        np.asarray(inputs["Wq"]), np.asarray(inputs["Wk"]),
        np.asarray(inputs["Wv"]), np.asarray(inputs["Wo"]))
    out_arrs, dev_in = run(in_maps)          # warmup + device_put
    times = []
    for _ in range(iters):
        t0 = time.perf_counter()
        out_arrs, _ = run(in_maps, device_inputs=dev_in)
        times.append((time.perf_counter() - t0) * 1e9)
    out = np.asarray(out_arrs[out_names.index("out")])
    return out, times


def bench_device(inputs, repeat=32, iters=4):
    """Estimate pure device execution time by running the NEFF `repeat`
    times inside one dispatch and differencing against a single run."""
    import time
    import jax

    if "nc" not in _CACHE:
        _CACHE["nc"] = build_nc()
    if "runner" not in _CACHE:
        _CACHE["runner"] = _make_runner(_CACHE["nc"])
    run, out_names, out_avals = _CACHE["runner"]
    in_maps = _prep_core_inputs(
        np.asarray(inputs["hidden_states"]),
        np.asarray(inputs["Wq"]), np.asarray(inputs["Wk"]),
        np.asarray(inputs["Wv"]), np.asarray(inputs["Wo"]))
    _, dev_in = run(in_maps)

    f1 = run.make_repeat(1)
    fR = run.make_repeat(repeat)
    jax.block_until_ready(f1(*dev_in))
    jax.block_until_ready(fR(*dev_in))

    def timeit(f):
        best = float("inf")
        for _ in range(iters):
            t0 = time.perf_counter()
            jax.block_until_ready(f(*dev_in))
            best = min(best, time.perf_counter() - t0)
        return best

    t1 = timeit(f1)
    tR = timeit(fR)
    per_iter_ns = (tR - t1) / (repeat - 1) * 1e9
    return per_iter_ns, t1 * 1e9, tR * 1e9


def kernel(hidden_states, Wq, bq, Wk, bk, Wv, bv, Wo, bo, counter, ucb,
           **extra):
    hidden_states = np.asarray(hidden_states)
    Wq, bq = np.asarray(Wq), np.asarray(bq)
    Wk, bk = np.asarray(Wk), np.asarray(bk)
    Wv, bv = np.asarray(Wv), np.asarray(bv)
    Wo, bo = np.asarray(Wo), np.asarray(bo)

    if np.any(bq) or np.any(bk):
        # exact numpy fallback (not expected to trigger: spec fills zeros)
        return _numpy_reference(hidden_states, Wq, bq, Wk, bk, Wv, bv, Wo, bo)

    if "nc" not in _CACHE:
        _CACHE["nc"] = build_nc()
    nc = _CACHE["nc"]
    if "runner" not in _CACHE:
        _CACHE["runner"] = _make_runner(nc)
    run, out_names, out_avals = _CACHE["runner"]

    in_maps = _prep_core_inputs(hidden_states, Wq, Wk, Wv, Wo)
    out_arrs, _ = run(in_maps)
    full = np.asarray(out_arrs[out_names.index("out")])
    out = full  # [NCORES*NB, T, C] — concat over cores is exactly batch order

    # bv/bo enter the output linearly: out += bv @ Wo + bo (attention rows sum
    # to one, so the bv term is constant across positions).
    if np.any(bv) or np.any(bo):
        out = out + (bv.astype(np.float64) @ Wo.astype(np.float64)
                     + bo.astype(np.float64)).astype(np.float32)[None, None, :]
    return out.astype(np.float32)


def _make_runner(nc):
    """Cached jitted runner (mirrors bass2jax.run_bass_via_pjrt) that keeps
    inputs device-resident so repeated calls time pure device execution."""
    import jax
    import concourse.mybir as mybir
    from concourse import bass2jax
    from concourse.bass2jax import _bass_exec_p, install_neuronx_cc_hook
    from jax.sharding import Mesh, PartitionSpec
    from jax.experimental.shard_map import shard_map

    install_neuronx_cc_hook()
    n_cores = NCORES
    partition_name = (nc.partition_id_tensor.name
                      if nc.partition_id_tensor else None)
    in_names, out_names, out_avals = [], [], []
    for alloc in nc.m.functions[0].allocations:
        if not isinstance(alloc, mybir.MemoryLocationSet):
            continue
        name = alloc.memorylocations[0].name
        if alloc.kind == "ExternalInput":
            if name != partition_name:
                in_names.append(name)
        elif alloc.kind == "ExternalOutput":
            shape = tuple(alloc.tensor_shape)
            dtype = mybir.dt.np(alloc.dtype)
            out_names.append(name)
            out_avals.append(jax.core.ShapedArray(shape, dtype))
    n_params = len(in_names)
    all_names = in_names + out_names
    if partition_name is not None:
        all_names = all_names + [partition_name]

    def _body(*args):
        operands = list(args)
        if partition_name is not None:
            operands.append(bass2jax.partition_id_tensor())
        outs = _bass_exec_p.bind(
            *operands,
            out_avals=tuple(out_avals),
            in_names=tuple(all_names),
            out_names=tuple(out_names),
            lowering_input_output_aliases=(),
            sim_require_finite=True,
            sim_require_nnan=True,
            nc=nc,
        )
        return tuple(outs)

    devices = jax.devices()[:n_cores]
    mesh = Mesh(np.asarray(devices), ("core",))
    in_specs = (PartitionSpec("core"),) * (n_params + len(out_names))
    out_specs = (PartitionSpec("core"),) * len(out_names)
    sharded = jax.jit(
        shard_map(_body, mesh=mesh, in_specs=in_specs, out_specs=out_specs,
                  check_rep=False),
        keep_unused=True,
    )

    def make_repeat(repeat):
        n_outs = len(out_names)

        def _body_r(*args):
            params = list(args[:n_params])
            outbufs = list(args[n_params:])
            outs = None
            for _ in range(repeat):
                # thread the previous iteration's outputs in as the output
                # operands: forces a data dependency so XLA cannot dedupe
                # or reorder the repeated effectful calls
                outs = _body(*params, *outbufs)
                outbufs = list(outs)
            return outs
        return jax.jit(
            shard_map(_body_r, mesh=mesh, in_specs=in_specs,
                      out_specs=out_specs, check_rep=False),
            keep_unused=True,
        )

    def run(in_maps, device_inputs=None):
        if device_inputs is None:
            concat_in = [
                np.concatenate([np.asarray(in_maps[c][nm]) for c in range(n_cores)],
                               axis=0)
                for nm in in_names
            ]
            concat_zeros = [
                np.zeros((n_cores * a.shape[0], *a.shape[1:]), a.dtype)
                for a in out_avals
            ]
            device_inputs = jax.device_put(
                concat_in + concat_zeros,
                [jax.sharding.NamedSharding(mesh, PartitionSpec("core"))]
                * (n_params + len(out_names)),
            )
        out_arrs = sharded(*device_inputs)
        jax.block_until_ready(out_arrs)
        return out_arrs, device_inputs

    run.make_repeat = make_repeat
    return run, out_names, out_avals


def _numpy_reference(hidden_states, Wq, bq, Wk, bk, Wv, bv, Wo, bo):
    x = hidden_states.astype(np.float64)
    q = (x @ Wq.astype(np.float64) + bq).reshape(B, T, H, D).transpose(0, 2, 1, 3)
    k = (x @ Wk.astype(np.float64) + bk).reshape(B, T, H, D).transpose(0, 2, 1, 3)
    v = (x @ Wv.astype(np.float64) + bv).reshape(B, T, H, D).transpose(0, 2, 1, 3)
    s = np.einsum("bhqd,bhkd->bhqk", q, k) / np.sqrt(D)
    causal = np.tril(np.ones((T, T), dtype=bool))
    s = np.where(causal, s, -np.inf)
    s = s - s.max(axis=-1, keepdims=True)
    p = np.exp(s)
    p = p / p.sum(axis=-1, keepdims=True)
    ctx = np.einsum("bhqk,bhkd->bhqd", p, v).transpose(0, 2, 1, 3).reshape(B, T, C)
    return (ctx @ Wo.astype(np.float64) + bo).astype(np.float32)

